# revision 23
# baseline (speedup 1.0000x reference)
"""Trainium2 Bass kernel for nn_DoublePSMCosineModule.

Math:
  cost_1[b,d,h,w] = mean_c(L[b,c,h,w] * R[b,c,h,w-d]),  d in [0,48)
  cost_2 same with R replaced by a fixed bilinear resample RS where
  row j of RS is built from columns x0(j), x0(j)+1 of R, upsampled
  96->320 along y by a constant sparse matrix Sy.
  out = concat([cost_1, cost_2], axis=1):  [4, 96, 96, 320] f32.

Device decomposition (per core = one (batch, H-half) pair, pure data
parallel, 8 cores):
  For each row j: cost rows are the 48 leading diagonals of the Gram
  band G1 = R_j^T L'_j (L' = L/512, folded on host), and for cost_2 of
  G2 = Sy^T Q_j with Q_j = t1_j^T L'_j where t1_j is the host-blended
  pair of R columns.  All device tensors are fp16 (the PE runs fp16 at
  1 cycle/row at any moving width, so the band windows shrink to
  128x176).  The band is covered by 3 window matmuls per Gram; windows
  are copied out of PSUM as two [64,112] half-window blocks (plus one
  [64,64] for the 64-row tail chunk) and the host extracts diagonals
  at gather time (pure re-indexing, no arithmetic).
"""

import json
import sys

import numpy as np

for _p in ("/opt/trn_rl_repo",):
    if _p not in sys.path:
        sys.path.insert(0, _p)

B, C, H, W, D = 4, 512, 96, 320, 48
NCORES = 8
JB = 48            # rows per core
NA = 10            # 32-row diagonal stair groups per Gram
SW = 80            # stair window width (host-side view)
CH = C // 128      # 4 c-chunks
NIN = 2 * CH * W + CH * H      # 2944 fp16 cols per j (L' | R | t1)
MROWS = (128, 128, 64)         # Gram row chunks
NWIN = (0, 128, 256)           # rhs window starts
NWID = (176, 176, 64)          # rhs window widths
PBASE = (0, 176, 352)          # window col base inside the PSUM tile
OCOLS = 512                    # shipped cols per j: 2*(112+112) + shared 64
JGRP = 8                       # j's per output DMA

_PROGRAM = None    # cached compiled Bass program


# ----------------------------------------------------------------- host tables
def _host_tables():
    j = np.arange(H)
    xpix = (((-1.0 + 2.0 * j.astype(np.float32) / np.float32(H)) + 1.0) * W - 1.0) / 2.0
    x0 = np.floor(xpix).astype(np.int64)
    wx1 = (xpix - x0).astype(np.float32)
    wx0 = (1.0 - wx1).astype(np.float32)
    vx0 = ((x0 >= 0) & (x0 < W)).astype(np.float32)
    vx1 = ((x0 + 1 >= 0) & (x0 + 1 < W)).astype(np.float32)

    k = np.arange(W)
    xvals = -1.0 + 2.0 * k.astype(np.float32) / np.float32(W) - 1.0 / np.float32(C)
    ypix = ((xvals + 1.0) * H - 1.0) / 2.0
    y0 = np.floor(ypix).astype(np.int64)
    wy1 = (ypix - y0).astype(np.float32)
    wy0 = (1.0 - wy1).astype(np.float32)
    Sy = np.zeros((H, W), dtype=np.float32)
    for kk in range(W):
        if 0 <= y0[kk] < H:
            Sy[y0[kk], kk] += wy0[kk]
        if 0 <= y0[kk] + 1 < H:
            Sy[y0[kk] + 1, kk] += wy1[kk]
    return x0, wx0, wx1, vx0, vx1, Sy


# ------------------------------------------------------------------ bir patch
def _fix_bir_json(raw: bytes) -> bytes:
    """walrus in this container rejects >1 sync wait per instruction;
    hoist extra waits onto preceding same-engine NoOps."""
    d = json.loads(raw)
    for fn in d["functions"]:
        for blk in fn["blocks"]:
            out = []
            for inst in blk["instructions"]:
                si = inst.get("sync_info")
                waits = (si or {}).get("on_wait") or []
                if len(waits) > 1:
                    for wi, w in enumerate(waits[:-1]):
                        out.append({
                            "debug": inst.get("debug"),
                            "engine": inst["engine"],
                            "ins": [],
                            "name": f"{inst['name']}-w{wi}",
                            "opcode": "NoOp",
                            "outs": [],
                            "sync_info": {"on_update": [], "on_wait": [w]},
                        })
                    si["on_wait"] = [waits[-1]]
                out.append(inst)
            blk["instructions"] = out
    return json.dumps(d).encode()


# ------------------------------------------------------------- device program
def _build_program():
    import concourse.bass as bass
    import concourse.mybir as mybir
    import concourse.tile as tile

    f16 = mybir.dt.float16
    f32 = mybir.dt.float32

    nc = bass.Bass("TRN2", target_bir_lowering=False, debug=False)
    lr = nc.dram_tensor("lr", [JB // 2, 128, 2 * NIN], f16, kind="ExternalInput").ap()
    syt = nc.dram_tensor("syt", [H, W], f16, kind="ExternalInput").ap()
    out2 = nc.dram_tensor("out2", [JB // JGRP, 128, JGRP * OCOLS], f16,
                          kind="ExternalOutput").ap()

    # half-window PSUM->SBUF copies: (psum part0, m, psum col0, width, dst col0)
    # dst layout per (cost,j): cost c m<2 at 224c + 112m; m2 shared at 448
    # (cost0 -> partitions 0-63, cost1 -> partitions 64-127).
    PAIRS = (
        (0, 0, 0, 112, 0),       # m0 stairs 0,1
        (64, 0, 64, 112, 0),     # m0 stairs 2,3
        (0, 1, 0, 112, 112),     # m1 stairs 0,1
        (64, 1, 64, 112, 112),   # m1 stairs 2,3
        (0, 2, 0, 64, 448),      # m2 stairs 0,1
    )

    with tile.TileContext(nc) as tc:
        with (
            tc.tile_pool(name="io", bufs=5) as io_pool,
            tc.tile_pool(name="aux", bufs=4) as aux_pool,
            tc.tile_pool(name="st", bufs=3) as st_pool,
            tc.tile_pool(name="const", bufs=1) as const_pool,
            tc.tile_pool(name="psa", bufs=3, space="PSUM") as psa_pool,
            tc.tile_pool(name="psb", bufs=2, space="PSUM") as psb_pool,
            tc.tile_pool(name="psq", bufs=3, space="PSUM") as psq_pool,
        ):
            sy_t = const_pool.tile([H, W], f16)
            nc.sync.dma_start(sy_t[:], syt[:])

            qs_hist = {}
            st_hist = {}

            def g2_and_cost1(jprev):
                """Emit G2 matmuls for row jprev and its cost-1 copies.
                The m2 chunk lands on PSUM partitions 64-127 so it shares
                output columns with cost-0's m2 (partitions 0-63)."""
                qsp = qs_hist.pop(jprev)
                pb = psb_pool.tile([128, 416], f32, tag="g2")
                for m in range(3):
                    p0 = 64 if m == 2 else 0
                    nc.tensor.matmul(
                        pb[p0:p0 + MROWS[m], PBASE[m]:PBASE[m] + NWID[m]],
                        lhsT=sy_t[:, 128 * m:128 * m + MROWS[m]],
                        rhs=qsp[:, NWIN[m]:NWIN[m] + NWID[m]],
                        start=True, stop=True,
                    )
                copy_windows(pb, st_hist[jprev], jprev, 1)

            def copy_windows(ps, stt, j, cost):
                """5 half-window copies PSUM f32 -> SBUF f16 on vector+scalar."""
                jb = (j % JGRP) * OCOLS
                for i, (p0, m, c0, wid, dc) in enumerate(PAIRS):
                    sp = 64 * cost if m == 2 else p0
                    d = stt[sp:sp + 64, jb + dc + (224 * cost if m < 2 else 0):][:, 0:wid]
                    s = ps[sp:sp + 64, PBASE[m] + c0:PBASE[m] + c0 + wid]
                    if (i + j + cost) % 2 == 0:
                        nc.vector.tensor_copy(d, s)
                    else:
                        nc.scalar.copy(d, s)

            for j in range(JB):
                if j % 2 == 0:
                    iot = io_pool.tile([128, 2 * NIN], f16, tag="lrt")
                    nc.sync.dma_start(iot[:], lr[j // 2])
                base = (j % 2) * NIN
                lt = iot[:, base:base + CH * W]
                rt = iot[:, base + CH * W:base + 2 * CH * W]
                rc = iot[:, base + 2 * CH * W:base + NIN]

                # ---- cost_1 Gram band:  G1 = R^T L'  (contraction over c)
                pa = psa_pool.tile([128, 416], f32, tag="g1")
                for m in range(3):
                    for cc in range(CH):
                        nc.tensor.matmul(
                            pa[0:MROWS[m], PBASE[m]:PBASE[m] + NWID[m]],
                            lhsT=rt[:, cc * W + 128 * m:cc * W + 128 * m + MROWS[m]],
                            rhs=lt[:, cc * W + NWIN[m]:cc * W + NWIN[m] + NWID[m]],
                            start=(cc == 0), stop=(cc == CH - 1),
                        )

                # ---- cost_2 stage 1:  Q = t1^T L'  (over c), qs = f16(Q)
                pq = psq_pool.tile([H, W], f32, tag="q")
                for cc in range(CH):
                    nc.tensor.matmul(
                        pq[:],
                        lhsT=rc[:, cc * H:(cc + 1) * H],
                        rhs=lt[:, cc * W:(cc + 1) * W],
                        start=(cc == 0), stop=(cc == CH - 1),
                    )
                qs = aux_pool.tile([H, W], f16, tag="qs")
                if j % 2 == 0:
                    nc.vector.tensor_copy(qs[:], pq[:])
                else:
                    nc.scalar.copy(qs[:], pq[:])
                qs_hist[j] = qs

                if j % JGRP == 0:
                    stt = st_pool.tile([128, JGRP * OCOLS], f16, tag="st")
                st_hist[j] = stt

                # ---- cost_2 stage 2 for the PREVIOUS row (keeps PE fed)
                if j > 0:
                    g2_and_cost1(j - 1)

                # ---- cost_1 window copies for this row
                copy_windows(pa, stt, j, 0)

                if j > 0 and (j - 1) % JGRP == JGRP - 1:
                    nc.scalar.dma_start(out2[(j - 1) // JGRP], st_hist[j - JGRP][:])

            g2_and_cost1(JB - 1)
            nc.scalar.dma_start(out2[JB // JGRP - 1], st_hist[JB - JGRP][:])

    raw = _fix_bir_json(nc.to_json_bytes())
    nc.to_json_bytes = lambda: raw
    return nc


# ------------------------------------------------------------------- host side
def _pack_core(left, right, core, tables):
    x0, wx0, wx1, vx0, vx1, Sy = tables
    b, half = core // 2, core % 2
    j0 = half * JB
    js = slice(j0, j0 + JB)

    Lb = left[b][:, js, :] * np.float32(1.0 / 512.0)   # [C, 48, W], mean folded
    Rb = right[b][:, js, :]
    # [48, 128(c_lo), 2, 4(c_hi), W] -> flat [48, 128, 2*CH*W]
    lrv = np.stack([Lb.reshape(CH, 128, JB, W), Rb.reshape(CH, 128, JB, W)])
    lrp = lrv.transpose(3, 2, 0, 1, 4).reshape(JB, 128, 2 * CH * W)

    jg = np.arange(j0, j0 + JB)
    c0 = np.clip(x0[jg], 0, W - 1)
    c1 = np.clip(x0[jg] + 1, 0, W - 1)
    w0 = (wx0 * vx0)[jg].astype(np.float32)
    w1 = (wx1 * vx1)[jg].astype(np.float32)
    Rfull = right[b]                                   # [C, H, W]
    t1 = Rfull[:, :, c0] * w0 + Rfull[:, :, c1] * w1   # [C, H, 48]
    t1 = t1.transpose(2, 0, 1).reshape(JB, CH, 128, H).transpose(0, 2, 1, 3)
    t1 = t1.reshape(JB, 128, CH * H)

    lrp = np.concatenate([lrp, t1], axis=2).astype(np.float16)
    lrp = np.ascontiguousarray(
        lrp.reshape(JB // 2, 2, 128, NIN).transpose(0, 2, 1, 3)
        .reshape(JB // 2, 128, 2 * NIN))

    return dict(lr=lrp, syt=Sy.astype(np.float16))


def _unshard(results):
    out = np.zeros((B, 2 * D, H, W), dtype=np.float32)
    # dst col base inside a per-(cost,j) 288-col block, for stair a
    for core in range(NCORES):
        b, half = core // 2, core % 2
        raw = results[core]["out2"].astype(np.float32)
        raw = raw.reshape(JB // JGRP, 128, JGRP, OCOLS)
        raw = raw.transpose(0, 2, 1, 3).reshape(JB, 128, OCOLS)
        o2 = np.zeros((2, JB, NA, 32, SW), dtype=np.float32)
        for c in range(2):
            for m in range(2):
                for s in range(4):
                    cb = 224 * c + 112 * m + 32 * s - (64 if s >= 2 else 0)
                    o2[c, :, 4 * m + s] = \
                        raw[:, 32 * s:32 * s + 32, cb:cb + SW]
            for s in range(2):
                wid = 64 - 32 * s
                o2[c, :, 8 + s, :, :wid] = \
                    raw[:, 64 * c + 32 * s:64 * c + 32 * s + 32,
                        448 + 32 * s:448 + 32 * s + wid]
        js = slice(half * JB, (half + 1) * JB)
        for d in range(D):
            diag = np.diagonal(o2, offset=d, axis1=3, axis2=4)   # [2, 48, 10, 32]
            for a in range(NA):
                w_lo = 32 * a + d
                n = min(w_lo + 32, W) - w_lo
                if n <= 0:
                    continue
                out[b, d, js, w_lo:w_lo + n] = diag[0, :, a, :n]
                out[b, D + d, js, w_lo:w_lo + n] = diag[1, :, a, :n]
    return out


def _ensure_axon_hooks():
    try:
        import antenv.axon_hooks  # noqa: F401
    except ImportError:
        import types
        import antenv
        m = types.ModuleType("antenv.axon_hooks")
        m._hook = None
        m.set_axon_ntff_profile_hook = lambda h: setattr(m, "_hook", h)
        m.get_axon_ntff_profile_hook = lambda: m._hook
        sys.modules["antenv.axon_hooks"] = m
        antenv.axon_hooks = m


def kernel(**inputs):
    global _PROGRAM
    _ensure_axon_hooks()
    from concourse.bass_utils import run_bass_kernel_spmd

    left = np.asarray(inputs["left_features"], dtype=np.float32)
    right = np.asarray(inputs["right_features"], dtype=np.float32)

    tables = _host_tables()
    in_maps = [_pack_core(left, right, core, tables) for core in range(NCORES)]

    if _PROGRAM is None:
        _PROGRAM = _build_program()
    res = run_bass_kernel_spmd(_PROGRAM, in_maps, list(range(NCORES)))
    global LAST_RESULT
    LAST_RESULT = res
    return _unshard(res.results)


LAST_RESULT = None


if __name__ == "__main__":
    rng = np.random.default_rng(0)
    li = rng.standard_normal((B, C, H, W), dtype=np.float32)
    ri = rng.standard_normal((B, C, H, W), dtype=np.float32)
    o = kernel(left_features=li, right_features=ri)
    print("kernel ran, out shape", o.shape, "finite:", np.isfinite(o).all())


# revision 24
# speedup vs baseline: 1.0474x; 1.0474x over previous
"""Trainium2 Bass kernel for nn_DoublePSMCosineModule.

Math:
  cost_1[b,d,h,w] = mean_c(L[b,c,h,w] * R[b,c,h,w-d]),  d in [0,48)
  cost_2 same with R replaced by a fixed bilinear resample RS where
  row j of RS is built from columns x0(j), x0(j)+1 of R, upsampled
  96->320 along y by a constant sparse matrix Sy.
  out = concat([cost_1, cost_2], axis=1):  [4, 96, 96, 320] f32.

Device decomposition (per core = one (batch, H-half) pair, pure data
parallel, 8 cores):
  For each row j: cost rows are the 48 leading diagonals of the Gram
  band G1 = R_j^T L'_j (L' = L/512, folded on host), and for cost_2 of
  G2 = Sy^T Q_j with Q_j = t1_j^T L'_j where t1_j is the host-blended
  pair of R columns.  All device tensors are fp16 (the PE runs fp16 at
  1 cycle/row at any moving width, so the band windows shrink to
  128x176).  The band is covered by 3 window matmuls per Gram; windows
  are copied out of PSUM as two [64,112] half-window blocks (plus one
  [64,64] for the 64-row tail chunk) and the host extracts diagonals
  at gather time (pure re-indexing, no arithmetic).
"""

import json
import sys

import numpy as np

for _p in ("/opt/trn_rl_repo",):
    if _p not in sys.path:
        sys.path.insert(0, _p)

B, C, H, W, D = 4, 512, 96, 320, 48
NCORES = 8
JB = 48            # rows per core
NA = 10            # 32-row diagonal stair groups per Gram
SW = 80            # stair window width (host-side view)
CH = C // 128      # 4 c-chunks
NIN = 2 * CH * W + CH * H      # 2944 fp16 cols per j (L' | R | t1)
MROWS = (128, 128, 64)         # Gram row chunks
NWIN = (0, 128, 256)           # rhs window starts
NWID = (176, 176, 64)          # rhs window widths
PBASE = (0, 176, 352)          # window col base inside the PSUM tile
OCOLS = 512                    # shipped cols per j: 2*(112+112) + shared 64
JGRP = 8                       # j's per output DMA

_PROGRAM = None    # cached compiled Bass program


# ----------------------------------------------------------------- host tables
def _host_tables():
    j = np.arange(H)
    xpix = (((-1.0 + 2.0 * j.astype(np.float32) / np.float32(H)) + 1.0) * W - 1.0) / 2.0
    x0 = np.floor(xpix).astype(np.int64)
    wx1 = (xpix - x0).astype(np.float32)
    wx0 = (1.0 - wx1).astype(np.float32)
    vx0 = ((x0 >= 0) & (x0 < W)).astype(np.float32)
    vx1 = ((x0 + 1 >= 0) & (x0 + 1 < W)).astype(np.float32)

    k = np.arange(W)
    xvals = -1.0 + 2.0 * k.astype(np.float32) / np.float32(W) - 1.0 / np.float32(C)
    ypix = ((xvals + 1.0) * H - 1.0) / 2.0
    y0 = np.floor(ypix).astype(np.int64)
    wy1 = (ypix - y0).astype(np.float32)
    wy0 = (1.0 - wy1).astype(np.float32)
    Sy = np.zeros((H, W), dtype=np.float32)
    for kk in range(W):
        if 0 <= y0[kk] < H:
            Sy[y0[kk], kk] += wy0[kk]
        if 0 <= y0[kk] + 1 < H:
            Sy[y0[kk] + 1, kk] += wy1[kk]
    return x0, wx0, wx1, vx0, vx1, Sy


# ------------------------------------------------------------------ bir patch
def _fix_bir_json(raw: bytes) -> bytes:
    """walrus in this container rejects >1 sync wait per instruction;
    hoist extra waits onto preceding same-engine NoOps."""
    d = json.loads(raw)
    for fn in d["functions"]:
        for blk in fn["blocks"]:
            out = []
            for inst in blk["instructions"]:
                si = inst.get("sync_info")
                waits = (si or {}).get("on_wait") or []
                if len(waits) > 1:
                    for wi, w in enumerate(waits[:-1]):
                        out.append({
                            "debug": inst.get("debug"),
                            "engine": inst["engine"],
                            "ins": [],
                            "name": f"{inst['name']}-w{wi}",
                            "opcode": "NoOp",
                            "outs": [],
                            "sync_info": {"on_update": [], "on_wait": [w]},
                        })
                    si["on_wait"] = [waits[-1]]
                out.append(inst)
            blk["instructions"] = out
    return json.dumps(d).encode()


# ------------------------------------------------------------- device program
def _build_program():
    import concourse.bass as bass
    import concourse.mybir as mybir
    import concourse.tile as tile

    f16 = mybir.dt.float16
    f32 = mybir.dt.float32

    nc = bass.Bass("TRN2", target_bir_lowering=False, debug=False)
    lr = nc.dram_tensor("lr", [JB // 2, 128, 2 * NIN], f16, kind="ExternalInput").ap()
    syt = nc.dram_tensor("syt", [H, W], f16, kind="ExternalInput").ap()
    out2 = nc.dram_tensor("out2", [JB // JGRP, 128, JGRP * OCOLS], f16,
                          kind="ExternalOutput").ap()

    # half-window PSUM->SBUF copies: (psum part0, m, psum col0, width, dst col0)
    # dst layout per (cost,j): cost c m<2 at 224c + 112m; m2 shared at 448
    # (cost0 -> partitions 0-63, cost1 -> partitions 64-127).
    PAIRS = (
        (0, 0, 0, 112, 0),       # m0 stairs 0,1
        (64, 0, 64, 112, 0),     # m0 stairs 2,3
        (0, 1, 0, 112, 112),     # m1 stairs 0,1
        (64, 1, 64, 112, 112),   # m1 stairs 2,3
        (0, 2, 0, 64, 448),      # m2 stairs 0,1
    )

    with tile.TileContext(nc) as tc:
        with (
            tc.tile_pool(name="io", bufs=5) as io_pool,
            tc.tile_pool(name="aux", bufs=4) as aux_pool,
            tc.tile_pool(name="st", bufs=3) as st_pool,
            tc.tile_pool(name="const", bufs=1) as const_pool,
            tc.tile_pool(name="psa", bufs=3, space="PSUM") as psa_pool,
            tc.tile_pool(name="psb", bufs=2, space="PSUM") as psb_pool,
            tc.tile_pool(name="psq", bufs=3, space="PSUM") as psq_pool,
        ):
            sy_t = const_pool.tile([H, W], f16)
            nc.sync.dma_start(sy_t[:], syt[:])

            qs_hist = {}
            st_hist = {}

            def g2_and_cost1(jprev):
                """Emit G2 matmuls for row jprev and its cost-1 copies.
                The m2 chunk lands on PSUM partitions 64-127 so it shares
                output columns with cost-0's m2 (partitions 0-63)."""
                qsp = qs_hist.pop(jprev)
                pb = psb_pool.tile([128, 416], f32, tag="g2")
                for m in range(3):
                    p0 = 64 if m == 2 else 0
                    nc.tensor.matmul(
                        pb[p0:p0 + MROWS[m], PBASE[m]:PBASE[m] + NWID[m]],
                        lhsT=sy_t[:, 128 * m:128 * m + MROWS[m]],
                        rhs=qsp[:, NWIN[m]:NWIN[m] + NWID[m]],
                        start=True, stop=True,
                    )
                copy_windows(pb, st_hist[jprev], jprev, 1)

            def copy_windows(ps, stt, j, cost):
                """5 half-window copies PSUM f32 -> SBUF f16 on vector+scalar."""
                jb = (j % JGRP) * OCOLS
                for i, (p0, m, c0, wid, dc) in enumerate(PAIRS):
                    sp = 64 * cost if m == 2 else p0
                    d = stt[sp:sp + 64, jb + dc + (224 * cost if m < 2 else 0):][:, 0:wid]
                    s = ps[sp:sp + 64, PBASE[m] + c0:PBASE[m] + c0 + wid]
                    if (i + j + cost) % 2 == 0:
                        nc.vector.tensor_copy(d, s)
                    else:
                        nc.scalar.copy(d, s)

            for j in range(JB):
                if j % 2 == 0:
                    iot = io_pool.tile([128, 2 * NIN], f16, tag="lrt")
                    nc.sync.dma_start(iot[:], lr[j // 2])
                base = (j % 2) * NIN
                lt = iot[:, base:base + CH * W]
                rt = iot[:, base + CH * W:base + 2 * CH * W]
                rc = iot[:, base + 2 * CH * W:base + NIN]

                # ---- cost_1 Gram band:  G1 = R^T L'  (contraction over c)
                pa = psa_pool.tile([128, 416], f32, tag="g1")
                for m in range(3):
                    for cc in range(CH):
                        nc.tensor.matmul(
                            pa[0:MROWS[m], PBASE[m]:PBASE[m] + NWID[m]],
                            lhsT=rt[:, cc * W + 128 * m:cc * W + 128 * m + MROWS[m]],
                            rhs=lt[:, cc * W + NWIN[m]:cc * W + NWIN[m] + NWID[m]],
                            start=(cc == 0), stop=(cc == CH - 1),
                        )

                # ---- cost_2 stage 1:  Q = t1^T L'  (over c), qs = f16(Q)
                pq = psq_pool.tile([H, W], f32, tag="q")
                for cc in range(CH):
                    nc.tensor.matmul(
                        pq[:],
                        lhsT=rc[:, cc * H:(cc + 1) * H],
                        rhs=lt[:, cc * W:(cc + 1) * W],
                        start=(cc == 0), stop=(cc == CH - 1),
                    )
                qs = aux_pool.tile([H, W], f16, tag="qs")
                if j % 2 == 0:
                    nc.vector.tensor_copy(qs[:], pq[:])
                else:
                    nc.scalar.copy(qs[:], pq[:])
                qs_hist[j] = qs

                if j % JGRP == 0:
                    stt = st_pool.tile([128, JGRP * OCOLS], f16, tag="st")
                st_hist[j] = stt

                # ---- cost_2 stage 2 for the PREVIOUS row (keeps PE fed)
                if j > 0:
                    g2_and_cost1(j - 1)

                # ---- cost_1 window copies for this row
                copy_windows(pa, stt, j, 0)

                if j > 0 and (j - 1) % JGRP == JGRP - 1:
                    nc.scalar.dma_start(out2[(j - 1) // JGRP], st_hist[j - JGRP][:])

            g2_and_cost1(JB - 1)
            nc.scalar.dma_start(out2[JB // JGRP - 1], st_hist[JB - JGRP][:])

    raw = _fix_bir_json(nc.to_json_bytes())
    nc.to_json_bytes = lambda: raw
    return nc


# ------------------------------------------------------------------- host side
def _pack_core(left, right, core, tables):
    x0, wx0, wx1, vx0, vx1, Sy = tables
    b, half = core // 2, core % 2
    j0 = half * JB
    js = slice(j0, j0 + JB)

    Lb = left[b][:, js, :] * np.float32(1.0 / 512.0)   # [C, 48, W], mean folded
    Rb = right[b][:, js, :]
    # [48, 128(c_lo), 2, 4(c_hi), W] -> flat [48, 128, 2*CH*W]
    lrv = np.stack([Lb.reshape(CH, 128, JB, W), Rb.reshape(CH, 128, JB, W)])
    lrp = lrv.transpose(3, 2, 0, 1, 4).reshape(JB, 128, 2 * CH * W)

    jg = np.arange(j0, j0 + JB)
    c0 = np.clip(x0[jg], 0, W - 1)
    c1 = np.clip(x0[jg] + 1, 0, W - 1)
    w0 = (wx0 * vx0)[jg].astype(np.float32)
    w1 = (wx1 * vx1)[jg].astype(np.float32)
    Rfull = right[b]                                   # [C, H, W]
    t1 = Rfull[:, :, c0] * w0 + Rfull[:, :, c1] * w1   # [C, H, 48]
    t1 = t1.transpose(2, 0, 1).reshape(JB, CH, 128, H).transpose(0, 2, 1, 3)
    t1 = t1.reshape(JB, 128, CH * H)

    lrp = np.concatenate([lrp, t1], axis=2).astype(np.float16)
    lrp = np.ascontiguousarray(
        lrp.reshape(JB // 2, 2, 128, NIN).transpose(0, 2, 1, 3)
        .reshape(JB // 2, 128, 2 * NIN))

    return dict(lr=lrp, syt=Sy.astype(np.float16))


def _unshard(results):
    out = np.zeros((B, 2 * D, H, W), dtype=np.float32)
    for core in range(NCORES):
        b, half = core // 2, core % 2
        raw = results[core]["out2"].astype(np.float32)
        raw = raw.reshape(JB // JGRP, 128, JGRP, OCOLS)
        raw = raw.transpose(0, 2, 1, 3).reshape(JB, 128, OCOLS)
        o2 = np.zeros((2, JB, NA, 32, SW), dtype=np.float32)
        for c in range(2):
            for m in range(2):
                for s in range(4):
                    cb = 224 * c + 112 * m + 32 * s - (64 if s >= 2 else 0)
                    o2[c, :, 4 * m + s] = \
                        raw[:, 32 * s:32 * s + 32, cb:cb + SW]
            for s in range(2):
                wid = 64 - 32 * s
                o2[c, :, 8 + s, :, :wid] = \
                    raw[:, 64 * c + 32 * s:64 * c + 32 * s + 32,
                        448 + 32 * s:448 + 32 * s + wid]
        js = slice(half * JB, (half + 1) * JB)
        for d in range(D):
            diag = np.diagonal(o2, offset=d, axis1=3, axis2=4)   # [2, 48, 10, 32]
            for a in range(NA):
                w_lo = 32 * a + d
                n = min(w_lo + 32, W) - w_lo
                if n <= 0:
                    continue
                out[b, d, js, w_lo:w_lo + n] = diag[0, :, a, :n]
                out[b, D + d, js, w_lo:w_lo + n] = diag[1, :, a, :n]
    return out


def _ensure_axon_hooks():
    try:
        import antenv.axon_hooks  # noqa: F401
    except ImportError:
        import types
        import antenv
        m = types.ModuleType("antenv.axon_hooks")
        m._hook = None
        m.set_axon_ntff_profile_hook = lambda h: setattr(m, "_hook", h)
        m.get_axon_ntff_profile_hook = lambda: m._hook
        sys.modules["antenv.axon_hooks"] = m
        antenv.axon_hooks = m


def kernel(**inputs):
    global _PROGRAM
    _ensure_axon_hooks()
    from concourse.bass_utils import run_bass_kernel_spmd

    left = np.asarray(inputs["left_features"], dtype=np.float32)
    right = np.asarray(inputs["right_features"], dtype=np.float32)

    tables = _host_tables()
    in_maps = [_pack_core(left, right, core, tables) for core in range(NCORES)]

    if _PROGRAM is None:
        _PROGRAM = _build_program()
    res = run_bass_kernel_spmd(_PROGRAM, in_maps, list(range(NCORES)))
    global LAST_RESULT
    LAST_RESULT = res
    return _unshard(res.results)


LAST_RESULT = None


if __name__ == "__main__":
    rng = np.random.default_rng(0)
    li = rng.standard_normal((B, C, H, W), dtype=np.float32)
    ri = rng.standard_normal((B, C, H, W), dtype=np.float32)
    o = kernel(left_features=li, right_features=ri)
    print("kernel ran, out shape", o.shape, "finite:", np.isfinite(o).all())


# revision 28
# speedup vs baseline: 1.1409x; 1.0893x over previous
"""Trainium2 Bass kernel for nn_DoublePSMCosineModule.

Math:
  cost_1[b,d,h,w] = mean_c(L[b,c,h,w] * R[b,c,h,w-d]),  d in [0,48)
  cost_2 same with R replaced by a fixed bilinear resample RS where
  row j of RS is built from columns x0(j), x0(j)+1 of R, upsampled
  96->320 along y by a constant sparse matrix Sy.
  out = concat([cost_1, cost_2], axis=1):  [4, 96, 96, 320] f32.

Device decomposition (per core = one (batch, H-half) pair, pure data
parallel, 8 cores):
  For each row j: cost rows are the 48 leading diagonals of the Gram
  band G1 = R_j^T L'_j (L' = L/512, folded on host), and for cost_2 of
  G2 = Sy^T Q_j with Q_j = t1_j^T L'_j where t1_j is the host-blended
  pair of R columns.  All device tensors are fp16 (the PE runs fp16 at
  1 cycle/row at any moving width, so the band windows shrink to
  128x176).  The band is covered by 3 window matmuls per Gram; windows
  are copied out of PSUM as two [64,112] half-window blocks (plus one
  [64,64] for the 64-row tail chunk) and the host extracts diagonals
  at gather time (pure re-indexing, no arithmetic).
"""

import json
import sys

import numpy as np

for _p in ("/opt/trn_rl_repo",):
    if _p not in sys.path:
        sys.path.insert(0, _p)

B, C, H, W, D = 4, 512, 96, 320, 48
NCORES = 8
JB = 48            # rows per core
NA = 10            # 32-row diagonal stair groups per Gram
SW = 80            # stair window width (host-side view)
CH = C // 128      # 4 c-chunks
NIN = 2 * CH * W + CH * H      # 2944 fp16 cols per j (L' | R | t1)
MROWS = (128, 128, 64)         # Gram row chunks
NWIN = (0, 128, 256)           # rhs window starts
NWID = (176, 176, 64)          # rhs window widths
PBASE = (0, 176, 352)          # window col base inside the PSUM tile
OCOLS = 512                    # shipped cols per j: 2*(112+112) + shared 64
JGRP = 8                       # j's per output DMA

_PROGRAM = None    # cached compiled Bass program


# ----------------------------------------------------------------- host tables
def _host_tables():
    j = np.arange(H)
    xpix = (((-1.0 + 2.0 * j.astype(np.float32) / np.float32(H)) + 1.0) * W - 1.0) / 2.0
    x0 = np.floor(xpix).astype(np.int64)
    wx1 = (xpix - x0).astype(np.float32)
    wx0 = (1.0 - wx1).astype(np.float32)
    vx0 = ((x0 >= 0) & (x0 < W)).astype(np.float32)
    vx1 = ((x0 + 1 >= 0) & (x0 + 1 < W)).astype(np.float32)

    k = np.arange(W)
    xvals = -1.0 + 2.0 * k.astype(np.float32) / np.float32(W) - 1.0 / np.float32(C)
    ypix = ((xvals + 1.0) * H - 1.0) / 2.0
    y0 = np.floor(ypix).astype(np.int64)
    wy1 = (ypix - y0).astype(np.float32)
    wy0 = (1.0 - wy1).astype(np.float32)
    Sy = np.zeros((H, W), dtype=np.float32)
    for kk in range(W):
        if 0 <= y0[kk] < H:
            Sy[y0[kk], kk] += wy0[kk]
        if 0 <= y0[kk] + 1 < H:
            Sy[y0[kk] + 1, kk] += wy1[kk]
    return x0, wx0, wx1, vx0, vx1, Sy


# ------------------------------------------------------------------ bir patch
def _fix_bir_json(raw: bytes) -> bytes:
    """walrus in this container rejects >1 sync wait per instruction;
    hoist extra waits onto preceding same-engine NoOps."""
    d = json.loads(raw)
    for fn in d["functions"]:
        for blk in fn["blocks"]:
            out = []
            for inst in blk["instructions"]:
                si = inst.get("sync_info")
                waits = (si or {}).get("on_wait") or []
                if len(waits) > 1:
                    for wi, w in enumerate(waits[:-1]):
                        out.append({
                            "debug": inst.get("debug"),
                            "engine": inst["engine"],
                            "ins": [],
                            "name": f"{inst['name']}-w{wi}",
                            "opcode": "NoOp",
                            "outs": [],
                            "sync_info": {"on_update": [], "on_wait": [w]},
                        })
                    si["on_wait"] = [waits[-1]]
                out.append(inst)
            blk["instructions"] = out
    return json.dumps(d).encode()


# ------------------------------------------------------------- device program
def _build_program():
    import concourse.bass as bass
    import concourse.mybir as mybir
    import concourse.tile as tile

    f16 = mybir.dt.float16
    f32 = mybir.dt.float32

    nc = bass.Bass("TRN2", target_bir_lowering=False, debug=False)
    lr = nc.dram_tensor("lr", [JB // 2, 128, 2 * NIN], f16, kind="ExternalInput").ap()
    syt = nc.dram_tensor("syt", [H, W], f16, kind="ExternalInput").ap()
    out2 = nc.dram_tensor("out2", [JB // JGRP, 128, JGRP * OCOLS], f16,
                          kind="ExternalOutput").ap()

    # dst layout per (cost,j): cost c m<2 at 224c + 112m; m2 shared at 448
    # (cost0 -> partitions 0-63, cost1 -> partitions 64-127).
    with tile.TileContext(nc) as tc:
        with (
            tc.tile_pool(name="io", bufs=5) as io_pool,
            tc.tile_pool(name="aux", bufs=4) as aux_pool,
            tc.tile_pool(name="st", bufs=3) as st_pool,
            tc.tile_pool(name="const", bufs=1) as const_pool,
            tc.tile_pool(name="psa", bufs=3, space="PSUM") as psa_pool,
            tc.tile_pool(name="psb", bufs=2, space="PSUM") as psb_pool,
            tc.tile_pool(name="psq", bufs=3, space="PSUM") as psq_pool,
        ):
            sy_t = const_pool.tile([H, W], f16)

            qs_hist = {}
            st_hist = {}

            def g2_and_cost1(jprev):
                """Emit G2 matmuls for row jprev and its cost-1 copies.
                The m2 chunk lands on PSUM partitions 64-127 so it shares
                output columns with cost-0's m2 (partitions 0-63)."""
                qsp = qs_hist.pop(jprev)
                pb = psb_pool.tile([128, 416], f32, tag="g2")
                for m in range(3):
                    p0 = 64 if m == 2 else 0
                    nc.tensor.matmul(
                        pb[p0:p0 + MROWS[m], PBASE[m]:PBASE[m] + NWID[m]],
                        lhsT=sy_t[:, 128 * m:128 * m + MROWS[m]],
                        rhs=qsp[:, NWIN[m]:NWIN[m] + NWID[m]],
                        start=True, stop=True,
                    )
                copy_windows(pb, st_hist[jprev], jprev, 1)

            def copy_windows(ps, stt, j, cost):
                """3 copies PSUM f32 -> SBUF f16 per cost on vector+scalar.
                The two 112-wide half-window reads of m0 and m1 sit 176
                apart in the PSUM tile; a hand-built 3-dim strided AP
                fetches both in one instruction."""
                jb = (j % JGRP) * OCOLS + 224 * cost
                for i, (sp, c0) in enumerate(((0, 0), (64, 64))):
                    d = stt[sp:sp + 64, jb:jb + 224]
                    s = ps[sp:sp + 64, c0:c0 + 112]
                    s.ap.insert(1, [176, 2])
                    if (i + j + cost) % 2 == 0:
                        nc.vector.tensor_copy(d, s)
                    else:
                        nc.scalar.copy(d, s)
                sp = 64 * cost
                d = stt[sp:sp + 64, (j % JGRP) * OCOLS + 448:][:, 0:64]
                s = ps[sp:sp + 64, 352:416]
                if (j + cost) % 2 == 0:
                    nc.scalar.copy(d, s)
                else:
                    nc.vector.tensor_copy(d, s)

            for j in range(JB):
                if j % 2 == 0:
                    iot = io_pool.tile([128, 2 * NIN], f16, tag="lrt")
                    nc.sync.dma_start(iot[:], lr[j // 2])
                if j == 0:
                    # issued after the first input pair so the stream's
                    # first fat packet isn't delayed behind the constant
                    nc.sync.dma_start(sy_t[:], syt[:])
                base = (j % 2) * NIN
                lt = iot[:, base:base + CH * W]
                rt = iot[:, base + CH * W:base + 2 * CH * W]
                rc = iot[:, base + 2 * CH * W:base + NIN]

                # ---- cost_1 Gram band:  G1 = R^T L'  (contraction over c)
                pa = psa_pool.tile([128, 416], f32, tag="g1")
                for m in range(3):
                    for cc in range(CH):
                        nc.tensor.matmul(
                            pa[0:MROWS[m], PBASE[m]:PBASE[m] + NWID[m]],
                            lhsT=rt[:, cc * W + 128 * m:cc * W + 128 * m + MROWS[m]],
                            rhs=lt[:, cc * W + NWIN[m]:cc * W + NWIN[m] + NWID[m]],
                            start=(cc == 0), stop=(cc == CH - 1),
                        )

                # ---- cost_2 stage 1:  Q = t1^T L'  (over c), qs = f16(Q)
                pq = psq_pool.tile([H, W], f32, tag="q")
                for cc in range(CH):
                    nc.tensor.matmul(
                        pq[:],
                        lhsT=rc[:, cc * H:(cc + 1) * H],
                        rhs=lt[:, cc * W:(cc + 1) * W],
                        start=(cc == 0), stop=(cc == CH - 1),
                    )
                qs = aux_pool.tile([H, W], f16, tag="qs")
                if j % 2 == 0:
                    nc.vector.tensor_copy(qs[:], pq[:])
                else:
                    nc.scalar.copy(qs[:], pq[:])
                qs_hist[j] = qs

                if j % JGRP == 0:
                    stt = st_pool.tile([128, JGRP * OCOLS], f16, tag="st")
                st_hist[j] = stt

                # ---- cost_2 stage 2 for the PREVIOUS row (keeps PE fed)
                if j > 0:
                    g2_and_cost1(j - 1)

                # ---- cost_1 window copies for this row
                copy_windows(pa, stt, j, 0)

                if j > 0 and (j - 1) % JGRP == JGRP - 1:
                    nc.scalar.dma_start(out2[(j - 1) // JGRP], st_hist[j - JGRP][:])

            g2_and_cost1(JB - 1)
            nc.scalar.dma_start(out2[JB // JGRP - 1], st_hist[JB - JGRP][:])

    raw = _fix_bir_json(nc.to_json_bytes())
    nc.to_json_bytes = lambda: raw
    return nc


# ------------------------------------------------------------------- host side
def _pack_core(left, right, core, tables):
    x0, wx0, wx1, vx0, vx1, Sy = tables
    b, half = core // 2, core % 2
    j0 = half * JB
    js = slice(j0, j0 + JB)

    Lb = left[b][:, js, :] * np.float32(1.0 / 512.0)   # [C, 48, W], mean folded
    Rb = right[b][:, js, :]
    # [48, 128(c_lo), 2, 4(c_hi), W] -> flat [48, 128, 2*CH*W]
    lrv = np.stack([Lb.reshape(CH, 128, JB, W), Rb.reshape(CH, 128, JB, W)])
    lrp = lrv.transpose(3, 2, 0, 1, 4).reshape(JB, 128, 2 * CH * W)

    jg = np.arange(j0, j0 + JB)
    c0 = np.clip(x0[jg], 0, W - 1)
    c1 = np.clip(x0[jg] + 1, 0, W - 1)
    w0 = (wx0 * vx0)[jg].astype(np.float32)
    w1 = (wx1 * vx1)[jg].astype(np.float32)
    Rfull = right[b]                                   # [C, H, W]
    t1 = Rfull[:, :, c0] * w0 + Rfull[:, :, c1] * w1   # [C, H, 48]
    t1 = t1.transpose(2, 0, 1).reshape(JB, CH, 128, H).transpose(0, 2, 1, 3)
    t1 = t1.reshape(JB, 128, CH * H)

    lrp = np.concatenate([lrp, t1], axis=2).astype(np.float16)
    lrp = np.ascontiguousarray(
        lrp.reshape(JB // 2, 2, 128, NIN).transpose(0, 2, 1, 3)
        .reshape(JB // 2, 128, 2 * NIN))

    return dict(lr=lrp, syt=Sy.astype(np.float16))


def _unshard(results):
    out = np.zeros((B, 2 * D, H, W), dtype=np.float32)
    for core in range(NCORES):
        b, half = core // 2, core % 2
        raw = results[core]["out2"].astype(np.float32)
        raw = raw.reshape(JB // JGRP, 128, JGRP, OCOLS)
        raw = raw.transpose(0, 2, 1, 3).reshape(JB, 128, OCOLS)
        o2 = np.zeros((2, JB, NA, 32, SW), dtype=np.float32)
        for c in range(2):
            for m in range(2):
                for s in range(4):
                    cb = 224 * c + 112 * m + 32 * s - (64 if s >= 2 else 0)
                    o2[c, :, 4 * m + s] = \
                        raw[:, 32 * s:32 * s + 32, cb:cb + SW]
            for s in range(2):
                wid = 64 - 32 * s
                o2[c, :, 8 + s, :, :wid] = \
                    raw[:, 64 * c + 32 * s:64 * c + 32 * s + 32,
                        448 + 32 * s:448 + 32 * s + wid]
        js = slice(half * JB, (half + 1) * JB)
        for d in range(D):
            diag = np.diagonal(o2, offset=d, axis1=3, axis2=4)   # [2, 48, 10, 32]
            for a in range(NA):
                w_lo = 32 * a + d
                n = min(w_lo + 32, W) - w_lo
                if n <= 0:
                    continue
                out[b, d, js, w_lo:w_lo + n] = diag[0, :, a, :n]
                out[b, D + d, js, w_lo:w_lo + n] = diag[1, :, a, :n]
    return out


def _ensure_axon_hooks():
    try:
        import antenv.axon_hooks  # noqa: F401
    except ImportError:
        import types
        import antenv
        m = types.ModuleType("antenv.axon_hooks")
        m._hook = None
        m.set_axon_ntff_profile_hook = lambda h: setattr(m, "_hook", h)
        m.get_axon_ntff_profile_hook = lambda: m._hook
        sys.modules["antenv.axon_hooks"] = m
        antenv.axon_hooks = m


def kernel(**inputs):
    global _PROGRAM
    _ensure_axon_hooks()
    from concourse.bass_utils import run_bass_kernel_spmd

    left = np.asarray(inputs["left_features"], dtype=np.float32)
    right = np.asarray(inputs["right_features"], dtype=np.float32)

    tables = _host_tables()
    in_maps = [_pack_core(left, right, core, tables) for core in range(NCORES)]

    if _PROGRAM is None:
        _PROGRAM = _build_program()
    res = run_bass_kernel_spmd(_PROGRAM, in_maps, list(range(NCORES)))
    global LAST_RESULT
    LAST_RESULT = res
    return _unshard(res.results)


LAST_RESULT = None


if __name__ == "__main__":
    rng = np.random.default_rng(0)
    li = rng.standard_normal((B, C, H, W), dtype=np.float32)
    ri = rng.standard_normal((B, C, H, W), dtype=np.float32)
    o = kernel(left_features=li, right_features=ri)
    print("kernel ran, out shape", o.shape, "finite:", np.isfinite(o).all())


# revision 33
# speedup vs baseline: 1.1554x; 1.0127x over previous
"""Trainium2 Bass kernel for nn_DoublePSMCosineModule.

Math:
  cost_1[b,d,h,w] = mean_c(L[b,c,h,w] * R[b,c,h,w-d]),  d in [0,48)
  cost_2 same with R replaced by a fixed bilinear resample RS where
  row j of RS is built from columns x0(j), x0(j)+1 of R, upsampled
  96->320 along y by a constant sparse matrix Sy.
  out = concat([cost_1, cost_2], axis=1):  [4, 96, 96, 320] f32.

Device decomposition (per core = one (batch, H-half) pair, pure data
parallel, 8 cores):
  For each row j: cost rows are the 48 leading diagonals of the Gram
  band G1 = R_j^T L'_j (L' = L/512, folded on host), and for cost_2 of
  G2 = Sy^T Q_j with Q_j = t1_j^T L'_j where t1_j is the host-blended
  pair of R columns.  All device tensors are fp16 (the PE runs fp16 at
  1 cycle/row at any moving width, so the band windows shrink to
  128x176).  The band is covered by 3 window matmuls per Gram; windows
  are copied out of PSUM as two [64,112] half-window blocks (plus one
  [64,64] for the 64-row tail chunk) and the host extracts diagonals
  at gather time (pure re-indexing, no arithmetic).
"""

import json
import sys

import numpy as np

for _p in ("/opt/trn_rl_repo",):
    if _p not in sys.path:
        sys.path.insert(0, _p)

B, C, H, W, D = 4, 512, 96, 320, 48
NCORES = 8
JB = 48            # rows per core
NA = 10            # 32-row diagonal stair groups per Gram
SW = 80            # stair window width (host-side view)
CH = C // 128      # 4 c-chunks
NIN = 2 * CH * W + CH * H      # 2944 fp16 cols per j (L' | R | t1)
MROWS = (128, 128, 64)         # Gram row chunks
NWIN = (0, 128, 256)           # rhs window starts
NWID = (176, 176, 64)          # rhs window widths
PBASE = (0, 176, 352)          # window col base inside the PSUM tile
OCOLS = 512                    # shipped cols per j: 2*(112+112) + shared 64
JGRP = 8                       # j's per output DMA

_PROGRAM = None    # cached compiled Bass program


# ----------------------------------------------------------------- host tables
def _host_tables():
    j = np.arange(H)
    xpix = (((-1.0 + 2.0 * j.astype(np.float32) / np.float32(H)) + 1.0) * W - 1.0) / 2.0
    x0 = np.floor(xpix).astype(np.int64)
    wx1 = (xpix - x0).astype(np.float32)
    wx0 = (1.0 - wx1).astype(np.float32)
    vx0 = ((x0 >= 0) & (x0 < W)).astype(np.float32)
    vx1 = ((x0 + 1 >= 0) & (x0 + 1 < W)).astype(np.float32)

    k = np.arange(W)
    xvals = -1.0 + 2.0 * k.astype(np.float32) / np.float32(W) - 1.0 / np.float32(C)
    ypix = ((xvals + 1.0) * H - 1.0) / 2.0
    y0 = np.floor(ypix).astype(np.int64)
    wy1 = (ypix - y0).astype(np.float32)
    wy0 = (1.0 - wy1).astype(np.float32)
    Sy = np.zeros((H, W), dtype=np.float32)
    for kk in range(W):
        if 0 <= y0[kk] < H:
            Sy[y0[kk], kk] += wy0[kk]
        if 0 <= y0[kk] + 1 < H:
            Sy[y0[kk] + 1, kk] += wy1[kk]
    return x0, wx0, wx1, vx0, vx1, Sy


# ------------------------------------------------------------------ bir patch
def _fix_bir_json(raw: bytes) -> bytes:
    """walrus in this container rejects >1 sync wait per instruction;
    hoist extra waits onto preceding same-engine NoOps."""
    d = json.loads(raw)
    for fn in d["functions"]:
        for blk in fn["blocks"]:
            out = []
            for inst in blk["instructions"]:
                si = inst.get("sync_info")
                waits = (si or {}).get("on_wait") or []
                if len(waits) > 1:
                    for wi, w in enumerate(waits[:-1]):
                        out.append({
                            "debug": inst.get("debug"),
                            "engine": inst["engine"],
                            "ins": [],
                            "name": f"{inst['name']}-w{wi}",
                            "opcode": "NoOp",
                            "outs": [],
                            "sync_info": {"on_update": [], "on_wait": [w]},
                        })
                    si["on_wait"] = [waits[-1]]
                out.append(inst)
            blk["instructions"] = out
    return json.dumps(d).encode()


# ------------------------------------------------------------- device program
def _build_program():
    import concourse.bass as bass
    import concourse.mybir as mybir
    import concourse.tile as tile

    f16 = mybir.dt.float16
    f32 = mybir.dt.float32

    nc = bass.Bass("TRN2", target_bir_lowering=False, debug=False)
    lr = nc.dram_tensor("lr", [JB // 2, 128, 2 * NIN], f16, kind="ExternalInput").ap()
    syt = nc.dram_tensor("syt", [H, W], f16, kind="ExternalInput").ap()
    out2 = nc.dram_tensor("out2", [JB // JGRP, 128, JGRP * OCOLS], f16,
                          kind="ExternalOutput").ap()

    # dst layout per (cost,j): cost c m<2 at 224c + 112m; m2 shared at 448
    # (cost0 -> partitions 0-63, cost1 -> partitions 64-127).
    with tile.TileContext(nc) as tc:
        with (
            tc.tile_pool(name="io", bufs=5) as io_pool,
            tc.tile_pool(name="iot", bufs=2) as iot_pool,
            tc.tile_pool(name="aux", bufs=4) as aux_pool,
            tc.tile_pool(name="st", bufs=3) as st_pool,
            tc.tile_pool(name="stt", bufs=2) as stt_pool,
            tc.tile_pool(name="const", bufs=1) as const_pool,
            tc.tile_pool(name="psa", bufs=3, space="PSUM") as psa_pool,
            tc.tile_pool(name="psb", bufs=2, space="PSUM") as psb_pool,
            tc.tile_pool(name="psq", bufs=3, space="PSUM") as psq_pool,
        ):
            sy_t = const_pool.tile([H, W], f16)

            qs_hist = {}
            st_hist = {}

            def g2_and_cost1(jprev):
                """Emit G2 matmuls for row jprev and its cost-1 copies.
                The m2 chunk lands on PSUM partitions 64-127 so it shares
                output columns with cost-0's m2 (partitions 0-63)."""
                qsp = qs_hist.pop(jprev)
                pb = psb_pool.tile([128, 416], f32, tag="g2")
                for m in range(3):
                    p0 = 64 if m == 2 else 0
                    nc.tensor.matmul(
                        pb[p0:p0 + MROWS[m], PBASE[m]:PBASE[m] + NWID[m]],
                        lhsT=sy_t[:, 128 * m:128 * m + MROWS[m]],
                        rhs=qsp[:, NWIN[m]:NWIN[m] + NWID[m]],
                        start=True, stop=True,
                    )
                copy_windows(pb, st_hist[jprev], jprev, 1)

            def copy_windows(ps, stt, j, cost):
                """3 copies PSUM f32 -> SBUF f16 per cost on vector+scalar.
                The two 112-wide half-window reads of m0 and m1 sit 176
                apart in the PSUM tile; a hand-built 3-dim strided AP
                fetches both in one instruction."""
                slot = (j % 4) if j >= JB - JGRP else (j % JGRP)
                jb = slot * OCOLS + 224 * cost
                for i, (sp, c0) in enumerate(((0, 0), (64, 64))):
                    d = stt[sp:sp + 64, jb:jb + 224]
                    s = ps[sp:sp + 64, c0:c0 + 112]
                    s.ap.insert(1, [176, 2])
                    if (i + j + cost) % 2 == 0:
                        nc.vector.tensor_copy(d, s)
                    else:
                        nc.scalar.copy(d, s)
                sp = 64 * cost
                d = stt[sp:sp + 64, slot * OCOLS + 448:][:, 0:64]
                s = ps[sp:sp + 64, 352:416]
                if (j + cost) % 2 == 0:
                    nc.scalar.copy(d, s)
                else:
                    nc.vector.tensor_copy(d, s)

            for j in range(JB):
                if j >= JB - 2:
                    # last two rows arrive as single-row DMAs so the final
                    # row's compute starts as soon as its bytes land
                    iot = iot_pool.tile([128, NIN], f16, tag="lrt_tail")
                    nc.sync.dma_start(
                        iot[:],
                        lr[JB // 2 - 1][:, (j % 2) * NIN:(j % 2 + 1) * NIN])
                    base = 0
                else:
                    if j % 2 == 0:
                        iot = io_pool.tile([128, 2 * NIN], f16, tag="lrt")
                        nc.sync.dma_start(iot[:], lr[j // 2])
                    if j == 0:
                        # issued after the first input pair so the stream's
                        # first fat packet isn't delayed behind the constant
                        nc.sync.dma_start(sy_t[:], syt[:])
                    base = (j % 2) * NIN
                lt = iot[:, base:base + CH * W]
                rt = iot[:, base + CH * W:base + 2 * CH * W]
                rc = iot[:, base + 2 * CH * W:base + NIN]

                # ---- cost_1 Gram band:  G1 = R^T L'  (contraction over c)
                pa = psa_pool.tile([128, 416], f32, tag="g1")
                for m in range(3):
                    for cc in range(CH):
                        nc.tensor.matmul(
                            pa[0:MROWS[m], PBASE[m]:PBASE[m] + NWID[m]],
                            lhsT=rt[:, cc * W + 128 * m:cc * W + 128 * m + MROWS[m]],
                            rhs=lt[:, cc * W + NWIN[m]:cc * W + NWIN[m] + NWID[m]],
                            start=(cc == 0), stop=(cc == CH - 1),
                        )

                # ---- cost_2 stage 1:  Q = t1^T L'  (over c), qs = f16(Q)
                pq = psq_pool.tile([H, W], f32, tag="q")
                for cc in range(CH):
                    nc.tensor.matmul(
                        pq[:],
                        lhsT=rc[:, cc * H:(cc + 1) * H],
                        rhs=lt[:, cc * W:(cc + 1) * W],
                        start=(cc == 0), stop=(cc == CH - 1),
                    )
                qs = aux_pool.tile([H, W], f16, tag="qs")
                if j % 2 == 0:
                    nc.vector.tensor_copy(qs[:], pq[:])
                else:
                    nc.scalar.copy(qs[:], pq[:])
                qs_hist[j] = qs

                if j >= JB - JGRP:
                    # final group split in two half tiles so the last
                    # output flush is half the size
                    if j % 4 == 0:
                        stt = stt_pool.tile([128, 4 * OCOLS], f16, tag="stt")
                elif j % JGRP == 0:
                    stt = st_pool.tile([128, JGRP * OCOLS], f16, tag="st")
                st_hist[j] = stt

                # ---- cost_2 stage 2 for the PREVIOUS row (keeps PE fed)
                if j > 0:
                    g2_and_cost1(j - 1)

                # ---- cost_1 window copies for this row
                copy_windows(pa, stt, j, 0)

                if j > 0 and (j - 1) % JGRP == JGRP - 1 and j < JB - 1:
                    nc.scalar.dma_start(out2[(j - 1) // JGRP], st_hist[j - JGRP][:])
                if j == JB - 3:
                    # first half of the final group (rows 40-43) complete
                    nc.scalar.dma_start(
                        out2[JB // JGRP - 1][:, 0:4 * OCOLS],
                        st_hist[JB - JGRP][:])

            g2_and_cost1(JB - 1)
            nc.scalar.dma_start(
                out2[JB // JGRP - 1][:, 4 * OCOLS:JGRP * OCOLS],
                st_hist[JB - 4][:])

    raw = _fix_bir_json(nc.to_json_bytes())
    nc.to_json_bytes = lambda: raw
    return nc


# ------------------------------------------------------------------- host side
def _pack_core(left, right, core, tables):
    x0, wx0, wx1, vx0, vx1, Sy = tables
    b, half = core // 2, core % 2
    j0 = half * JB
    js = slice(j0, j0 + JB)

    Lb = left[b][:, js, :] * np.float32(1.0 / 512.0)   # [C, 48, W], mean folded
    Rb = right[b][:, js, :]
    # [48, 128(c_lo), 2, 4(c_hi), W] -> flat [48, 128, 2*CH*W]
    lrv = np.stack([Lb.reshape(CH, 128, JB, W), Rb.reshape(CH, 128, JB, W)])
    lrp = lrv.transpose(3, 2, 0, 1, 4).reshape(JB, 128, 2 * CH * W)

    jg = np.arange(j0, j0 + JB)
    c0 = np.clip(x0[jg], 0, W - 1)
    c1 = np.clip(x0[jg] + 1, 0, W - 1)
    w0 = (wx0 * vx0)[jg].astype(np.float32)
    w1 = (wx1 * vx1)[jg].astype(np.float32)
    Rfull = right[b]                                   # [C, H, W]
    t1 = Rfull[:, :, c0] * w0 + Rfull[:, :, c1] * w1   # [C, H, 48]
    t1 = t1.transpose(2, 0, 1).reshape(JB, CH, 128, H).transpose(0, 2, 1, 3)
    t1 = t1.reshape(JB, 128, CH * H)

    lrp = np.concatenate([lrp, t1], axis=2).astype(np.float16)
    lrp = np.ascontiguousarray(
        lrp.reshape(JB // 2, 2, 128, NIN).transpose(0, 2, 1, 3)
        .reshape(JB // 2, 128, 2 * NIN))

    return dict(lr=lrp, syt=Sy.astype(np.float16))


def _unshard(results):
    out = np.zeros((B, 2 * D, H, W), dtype=np.float32)
    for core in range(NCORES):
        b, half = core // 2, core % 2
        raw = results[core]["out2"].astype(np.float32)
        raw = raw.reshape(JB // JGRP, 128, JGRP, OCOLS)
        raw = raw.transpose(0, 2, 1, 3).reshape(JB, 128, OCOLS)
        o2 = np.zeros((2, JB, NA, 32, SW), dtype=np.float32)
        for c in range(2):
            for m in range(2):
                for s in range(4):
                    cb = 224 * c + 112 * m + 32 * s - (64 if s >= 2 else 0)
                    o2[c, :, 4 * m + s] = \
                        raw[:, 32 * s:32 * s + 32, cb:cb + SW]
            for s in range(2):
                wid = 64 - 32 * s
                o2[c, :, 8 + s, :, :wid] = \
                    raw[:, 64 * c + 32 * s:64 * c + 32 * s + 32,
                        448 + 32 * s:448 + 32 * s + wid]
        js = slice(half * JB, (half + 1) * JB)
        for d in range(D):
            diag = np.diagonal(o2, offset=d, axis1=3, axis2=4)   # [2, 48, 10, 32]
            for a in range(NA):
                w_lo = 32 * a + d
                n = min(w_lo + 32, W) - w_lo
                if n <= 0:
                    continue
                out[b, d, js, w_lo:w_lo + n] = diag[0, :, a, :n]
                out[b, D + d, js, w_lo:w_lo + n] = diag[1, :, a, :n]
    return out


def _ensure_axon_hooks():
    try:
        import antenv.axon_hooks  # noqa: F401
    except ImportError:
        import types
        import antenv
        m = types.ModuleType("antenv.axon_hooks")
        m._hook = None
        m.set_axon_ntff_profile_hook = lambda h: setattr(m, "_hook", h)
        m.get_axon_ntff_profile_hook = lambda: m._hook
        sys.modules["antenv.axon_hooks"] = m
        antenv.axon_hooks = m


def kernel(**inputs):
    global _PROGRAM
    _ensure_axon_hooks()
    from concourse.bass_utils import run_bass_kernel_spmd

    left = np.asarray(inputs["left_features"], dtype=np.float32)
    right = np.asarray(inputs["right_features"], dtype=np.float32)

    tables = _host_tables()
    in_maps = [_pack_core(left, right, core, tables) for core in range(NCORES)]

    if _PROGRAM is None:
        _PROGRAM = _build_program()
    res = run_bass_kernel_spmd(_PROGRAM, in_maps, list(range(NCORES)))
    global LAST_RESULT
    LAST_RESULT = res
    return _unshard(res.results)


LAST_RESULT = None


if __name__ == "__main__":
    rng = np.random.default_rng(0)
    li = rng.standard_normal((B, C, H, W), dtype=np.float32)
    ri = rng.standard_normal((B, C, H, W), dtype=np.float32)
    o = kernel(left_features=li, right_features=ri)
    print("kernel ran, out shape", o.shape, "finite:", np.isfinite(o).all())
